# revision 26
# baseline (speedup 1.0000x reference)
"""GQA causal-attention prefill kernel for 8 Trainium2 NeuronCores.

Sharding: core c -> (batch b = c//4, kv head g = c%4).
Replica groups [[0,1,2,3],[4,5,6,7]] (one per batch).

v3: attention head-blocks interleaved into the projection phase so the
per-head-group AllGathers fire ~80us earlier and the serial CC-ring
chain (~40us per 2-head AG, ~62 GB/s) hides under remaining projection
chains + later attention + o_proj waves. fp16 operands everywhere
(half DMA + SBUF traffic at the same 1 cycle/row PE rate), causal mask
folded into the S PSUM accumulation group as an identity-weights
matmul, v transposed via DMA XBAR (no PE/PSUM), rotating q/wq buffers
to fit SBUF, DMA queues split by role (sync=input stream, vector=og
writes, scalar=wo/yt, gpsimd=oag unpack), fp16 output.

Per-core program order:
  k(n0),k(n1),v(n0),v(n1),q0,q1, [attn h0], q2, [attn h1], q3,
  [attn h2], q4, [attn h3], q5, [attn h4], q6, [attn h5], [attn h6],
  o_proj waves in AG arrival order {h0},{h1,h2},{h3,h4},{h5,h6}.
Output per core: y[b][:, 896g:896(g+1)].T in fp16, host concatenates.
"""
import sys

if '/opt/trn_rl_repo' not in sys.path:
    sys.path.insert(0, '/opt/trn_rl_repo')

import ml_dtypes
import numpy as np

B, T, D = 2, 1024, 3584
NUM_HEADS, HEAD_DIM, NUM_KV = 28, 128, 4
REP = NUM_HEADS // NUM_KV            # 7
ROPE_THETA = 1000000.0
MASK_VAL = -30000.0                  # fp16-representable; *SCALE then exp -> 0
SCALE = HEAD_DIM ** -0.5
GROUP = 4                            # tensor-parallel group size (kv heads)
NCORES = 8
DK = D // 128                        # 28 contraction chunks over D
NT = T // 512                        # token 512-tiles
SK = T // 128                        # key 128-chunks
RG = [[0, 1, 2, 3], [4, 5, 6, 7]]
AGH = [(0, 1), (1, 3), (3, 5), (5, 6), (6, 7)]  # AllGather head groups [lo, hi)
WOA_HEADS = 3              # heads 0..2 (waves 1-2) get early wo weights

_CACHE = {}


def _build_nc():
    """Build the SPMD Bass program (same program on all 8 cores)."""
    import concourse.tile as tile
    from concourse import bacc, mybir
    from concourse.masks import make_identity

    FP32 = mybir.dt.float32
    FP16 = mybir.dt.float16
    Exp = mybir.ActivationFunctionType.Exp
    Ident = mybir.ActivationFunctionType.Identity
    mult = mybir.AluOpType.mult
    addop = mybir.AluOpType.add

    nc = bacc.Bacc("TRN2", target_bir_lowering=False, debug=False, num_devices=NCORES)

    xt = nc.dram_tensor("xt", [D, T], FP16, kind="ExternalInput")
    wq = nc.dram_tensor("wq", [D, REP * 128], FP16, kind="ExternalInput")
    wk = nc.dram_tensor("wk", [D, 128], FP16, kind="ExternalInput")
    wv = nc.dram_tensor("wv", [D, 128], FP16, kind="ExternalInput")
    wo = nc.dram_tensor("wo", [D, REP * 128], FP16, kind="ExternalInput")
    bqkv = nc.dram_tensor("bqkv", [REP + 2, 128], FP32, kind="ExternalInput")
    sincat = nc.dram_tensor("sincat", [128, T], FP16, kind="ExternalInput")
    coscat = nc.dram_tensor("coscat", [128, T], FP16, kind="ExternalInput")
    umask = nc.dram_tensor("umask", [128, 128], FP16, kind="ExternalInput")
    onescol = nc.dram_tensor("onescol", [128, 1], FP16, kind="ExternalInput")
    onesrow = nc.dram_tensor("onesrow", [1, 128], FP16, kind="ExternalInput")
    yt = nc.dram_tensor("yt", [REP * 128, T], FP16, kind="ExternalOutput")

    with tile.TileContext(nc) as tc:
        with (
            tc.tile_pool(name="consts", bufs=1) as consts,
            tc.tile_pool(name="qkv", bufs=1) as qkv,
            tc.tile_pool(name="qp", bufs=3) as qp,
            tc.tile_pool(name="dram", bufs=1, space="DRAM") as dram,
            tc.tile_pool(name="ep", bufs=3) as ep,
            tc.tile_pool(name="otp", bufs=1) as otp,
        ):
            ones_col = consts.tile([128, 1], FP16, tag="onescol")
            ones_row = consts.tile([1, 128], FP16, tag="onesrow")
            bias_sb = consts.tile([128, REP + 2], FP32, tag="bias")
            umask_sb = consts.tile([128, 128], FP16, tag="umask")
            id_sb = consts.tile([128, 128], FP16, tag="ident")
            make_identity(nc, id_sb[:])
            nc.scalar.dma_start(ones_col[:], onescol[:])
            nc.scalar.dma_start(ones_row[:], onesrow[:])
            nc.scalar.dma_start(bias_sb[:], bqkv.rearrange("m p -> p m"))
            nc.scalar.dma_start(umask_sb[:], umask[:])

            k_sb = qkv.tile([128, T], FP16, tag="k")
            vn_sb = qkv.tile([128, SK, 128], FP16, tag="vn")
            otf = otp.tile([128, NUM_HEADS, T], FP16, tag="otf")

            # head-group DRAM blocks for the pipelined AllGather (fp16)
            og = [dram.tile([(hi - lo) * 128, T], FP16, tag=f"og{i}", name=f"og{i}")
                  for i, (lo, hi) in enumerate(AGH)]
            oag = [dram.tile([GROUP * (hi - lo) * 128, T], FP16,
                             tag=f"oag{i}", name=f"oag{i}")
                   for i, (lo, hi) in enumerate(AGH)]

            # warmup collective: absorbs first-op CC-stream setup cost
            wu_in = dram.tile([128, 16], FP16, tag="wuin", name="wuin")
            wu_out = dram.tile([GROUP * 128, 16], FP16, tag="wuout", name="wuout")
            wu_sb = consts.tile([128, 16], FP16, tag="wusb")
            nc.vector.memset(wu_sb[:], 0.0)
            nc.gpsimd.dma_start(wu_in[:], wu_sb[:])
            nc.gpsimd.collective_compute(
                "AllGather",
                mybir.AluOpType.bypass,
                replica_groups=RG,
                ins=[wu_in[:].opt()],
                outs=[wu_out[:].opt()],
            )

            # early o_proj weights: heads 0..WOA_HEADS-1 (all 4 gp), loaded
            # during attention so waves 1-2 never wait; rest after x frees
            woa_ctx = tc.tile_pool(name="woa", bufs=1)
            woa = woa_ctx.__enter__()
            woa_sb = woa.tile([128, WOA_HEADS, GROUP, REP * 128], FP16, tag="woa")
            wor = wo.rearrange("(c p) n -> p c n", p=128)

            # ================= interleaved phase 1 + 4 ===================
            xp_ctx = tc.tile_pool(name="xp", bufs=1)
            xp = xp_ctx.__enter__()
            wqp_ctx = tc.tile_pool(name="wqp", bufs=2)
            wqp = wqp_ctx.__enter__()
            pp1_ctx = tc.tile_pool(name="pp1", bufs=1, space="PSUM")
            pp1 = pp1_ctx.__enter__()
            ropep_ctx = tc.tile_pool(name="ropep", bufs=2)
            ropep = ropep_ctx.__enter__()
            ppatt_ctx = None
            ppatt = None

            sin_sb = xp.tile([128, T], FP16, tag="sin")
            cos_sb = xp.tile([128, T], FP16, tag="cos")
            v_sb = xp.tile([128, T], FP16, tag="v")
            x_sb = xp.tile([128, DK, T], FP16, tag="x")
            wk_sb = xp.tile([128, DK, 128], FP16, tag="wk")
            wv_sb = xp.tile([128, DK, 128], FP16, tag="wv")

            xr = xt.rearrange("(c p) t -> p c t", p=128)
            wqr = wq.rearrange("(c p) n -> p c n", p=128)
            wkr = wk.rearrange("(c p) n -> p c n", p=128)
            wvr = wv.rearrange("(c p) n -> p c n", p=128)

            # priority-ordered input stream: x + k/v weights interleaved
            # finely on the sync queue (first chain fed within ~4us), q
            # weights + rope tables on the scalar queue in parallel
            for quarter in range(4):
                csl = slice(7 * quarter, 7 * quarter + 7)
                nc.sync.dma_start(wk_sb[:, csl, :], wkr[:, csl, :])
                nc.sync.dma_start(x_sb[:, csl, 0:512], xr[:, csl, 0:512])
            nc.sync.dma_start(wv_sb[:], wvr[:])
            for quarter in range(4):
                sl = (slice(None), slice(7 * quarter, 7 * quarter + 7),
                      slice(512, 1024))
                nc.sync.dma_start(x_sb[sl], xr[sl])
            nc.scalar.dma_start(sin_sb[:], sincat[:])
            nc.scalar.dma_start(cos_sb[:], coscat[:])

            wq_tiles = {}

            def load_wq(h, eng=None):
                wt = wqp.tile([128, DK, 128], FP16, tag="wqh", name=f"wq_{h}")
                (eng or nc.sync).dma_start(wt[:], wqr[:, :, 128 * h:128 * (h + 1)])
                wq_tiles[h] = wt

            load_wq(0, nc.scalar)
            load_wq(1, nc.scalar)

            def rope(X_full, n):
                X = X_full[:, 512 * n:512 * (n + 1)]
                tmp = ropep.tile([128, 512], FP16, tag="ropetmp")
                nc.vector.tensor_copy(tmp[0:64, :], X[64:128, :])
                nc.vector.tensor_copy(tmp[64:128, :], X[0:64, :])
                ssl = (slice(None), slice(512 * n, 512 * (n + 1)))
                nc.vector.tensor_tensor(tmp[:], tmp[:], sin_sb[ssl], op=mult)
                nc.vector.tensor_tensor(X, X, cos_sb[ssl], op=mult)
                nc.vector.tensor_tensor(X, X, tmp[:], op=addop)

            def chain(wsl3, dst, bi, n):
                """One projection chain: dst[:,512n:+512] = (w.T @ x) + bias."""
                ps = pp1.tile([128, 512], FP32, tag="proj", name=f"proj_{bi}_{n}")
                for kc in range(DK):
                    nc.tensor.matmul(
                        ps[:],
                        wsl3[:, kc, :],
                        x_sb[:, kc, 512 * n:512 * (n + 1)],
                        start=(kc == 0),
                        stop=(kc == DK - 1),
                    )
                nc.scalar.activation(
                    dst[:, 512 * n:512 * (n + 1)], ps[:], Ident,
                    bias=bias_sb[:, bi:bi + 1], scale=1.0,
                )

            # ---- k, v projections (+rope / PE transposes) ----
            # transposes use a transient PSUM pool that closes before the
            # attention pool opens (PSUM stays within 8 banks)
            trp_ctx = tc.tile_pool(name="trp", bufs=2, space="PSUM")
            trp = trp_ctx.__enter__()
            # n-interleaved so the n=0 work rides out the x n=1 DMA arrival
            for n in range(NT):
                chain(wk_sb, k_sb, 7, n)
                rope(k_sb, n)
                chain(wv_sb, v_sb, 8, n)
                for sc in range(4 * n, 4 * n + 4):
                    tp = trp.tile([128, 128], FP16, tag="tr", name=f"tr_{sc}")
                    nc.tensor.transpose(
                        tp[:], v_sb[:, 128 * sc:128 * (sc + 1)], id_sb[:]
                    )
                    nc.scalar.copy(vn_sb[:, sc, :], tp[:])
            trp_ctx.__exit__(None, None, None)
            ppatt_ctx = tc.tile_pool(name="ppatt", bufs=1, space="PSUM")
            ppatt = ppatt_ctx.__enter__()
            # early wo stream (sync queue)
            for hh in range(WOA_HEADS):
                for gp in range(GROUP):
                    nc.sync.dma_start(
                        woa_sb[:, hh, gp, :], wor[:, 7 * gp + hh, :]
                    )

            q_tiles = {}

            def qchain(h):
                qt = qp.tile([128, T], FP16, tag="q", name=f"q_{h}")
                for n in range(NT):
                    chain(wq_tiles[h], qt, h, n)
                    rope(qt, n)
                del wq_tiles[h]
                if h + 2 < REP:
                    load_wq(h + 2)
                q_tiles[h] = qt

            # ---- attention block for one head ----
            pending = []

            def finalize(h, tau, den, ops):
                rec = ep.tile([1, 512], FP32, tag="rec", name=f"rec_{h}_{tau}")
                nc.vector.reciprocal_approx_fast(rec[:], den[0:1, :])
                rec16 = ep.tile([1, 512], FP16, tag="rec16", name=f"rec16_{h}_{tau}")
                nc.vector.tensor_copy(rec16[:], rec[:])
                # partition-broadcast 1/den via a DRAM bounce (no PE/ACT work)
                recd = dram.tile([1, 512], FP16, tag="recd",
                                 name=f"recd_{h}_{tau}", bufs=2)
                nc.sync.dma_start(recd[:], rec16[:])
                bcs = ep.tile([128, 512], FP16, tag="bcs", name=f"bcs_{h}_{tau}")
                nc.sync.dma_start(bcs[:], recd[0:1, :].partition_broadcast(128))
                ost = ep.tile([128, 512], FP16, tag="ost", name=f"ost_{h}_{tau}")
                nc.vector.tensor_tensor(ost[:], ops[:], bcs[:], op=mult)
                grp = next(i for i, (lo, hi) in enumerate(AGH) if lo <= h < hi)
                lo, hi = AGH[grp]
                nc.sync.dma_start(
                    og[grp][128 * (h - lo):128 * (h - lo + 1),
                            512 * tau:512 * (tau + 1)],
                    ost[:],
                )
                if tau == NT - 1 and h == hi - 1:
                    nc.gpsimd.collective_compute(
                        "AllGather",
                        mybir.AluOpType.bypass,
                        replica_groups=RG,
                        ins=[og[grp][:].opt()],
                        outs=[oag[grp][:].opt()],
                    )
                    nh = hi - lo
                    for hh in range(lo, hi):
                        for gp in range(GROUP):
                            r0 = nh * 128 * gp + 128 * (hh - lo)
                            nc.gpsimd.dma_start(
                                otf[:, 7 * gp + hh, :],
                                oag[grp][r0:r0 + 128, :],
                            )

            def attn(h):
                qt = q_tiles.pop(h)
                for tau in range(NT):
                    n_sc = 4 * (tau + 1)
                    den = ppatt.tile([1, 512], FP32, tag="den",
                                     name=f"den_{h}_{tau}")
                    ops = ppatt.tile([128, 512], FP32, tag=f"opv{tau % 2}",
                                     name=f"ops_{h}_{tau}")
                    esum = ep.tile([128, 512], FP16, tag="esum",
                                   name=f"esum_{h}_{tau}")
                    etiles = {}

                    def emit_s(c):
                        delta = 128 * c - 512 * tau
                        t0 = max(delta, 0)
                        w = 512 - t0
                        sps = ppatt.tile([128, 512], FP32, tag=f"s{c % 4}",
                                         name=f"sps_{h}_{tau}_{c}")
                        tsl = slice(512 * tau + t0, 512 * (tau + 1))
                        nc.tensor.matmul(
                            sps[:, 0:w],
                            k_sb[:, 128 * c:128 * (c + 1)],
                            qt[:, tsl],
                            start=True,
                            stop=(delta < 0),
                            skip_group_check=True,
                        )
                        if delta >= 0:
                            # causal mask add on the diagonal 128 block:
                            # id.T @ umask == umask, accumulated in-group
                            nc.tensor.matmul(
                                sps[:, 0:128],
                                id_sb[:],
                                umask_sb[:],
                                start=False,
                                stop=True,
                                skip_group_check=True,
                            )
                        et = ep.tile([128, 512], FP16, tag="e",
                                     name=f"et_{h}_{tau}_{c}")
                        nc.scalar.activation(et[:, 0:w], sps[:, 0:w], Exp, scale=SCALE)
                        etiles[c] = (et, t0, w)

                    def emit_acc(c):
                        et, t0, w = etiles.pop(c)
                        # E column-sum accumulates on DVE (fp16 2x) instead
                        # of a PE ones-matmul chain
                        if c == 0:
                            nc.vector.tensor_copy(esum[:], et[:])
                        else:
                            nc.vector.tensor_tensor(
                                esum[:, t0:512], esum[:, t0:512], et[:, 0:w],
                                op=addop,
                            )
                        nc.tensor.matmul(
                            ops[:, t0:512], vn_sb[:, c, :], et[:, 0:w],
                            start=(c == 0), stop=(c == n_sc - 1),
                        )

                    LOOKAHEAD = 3 if n_sc > 4 else 2
                    for c in range(n_sc):
                        emit_s(c)
                        if c == LOOKAHEAD and pending:
                            finalize(*pending.pop(0))
                        if c >= LOOKAHEAD:
                            emit_acc(c - LOOKAHEAD)
                    for c in range(max(0, n_sc - LOOKAHEAD), n_sc):
                        emit_acc(c)
                    # single PE matmul turns esum into the softmax denominator
                    nc.tensor.matmul(
                        den[0:1, :], ones_col[:], esum[:], start=True, stop=True
                    )
                    pending.append((h, tau, den, ops))

            # ---- interleaved schedule ----
            qchain(0)
            qchain(1)
            for h in range(REP):
                attn(h)
                if h + 2 < REP:
                    qchain(h + 2)
                if pending:
                    finalize(*pending.pop(0))
            while pending:
                finalize(*pending.pop(0))

            ppatt_ctx.__exit__(None, None, None)
            ropep_ctx.__exit__(None, None, None)
            pp1_ctx.__exit__(None, None, None)
            wqp_ctx.__exit__(None, None, None)
            xp_ctx.__exit__(None, None, None)

            # ---- Phase 6: o_proj as per-AG-wave partial sums ------------
            with (
                tc.tile_pool(name="wob", bufs=1) as wob,
                tc.tile_pool(name="yaccp", bufs=1) as yaccp,
                tc.tile_pool(name="pp6", bufs=3, space="PSUM") as pp6,
            ):
                nwob = REP - WOA_HEADS
                wob_sb = wob.tile([128, nwob, GROUP, REP * 128], FP16, tag="wob")
                # late wo stream (sync queue, starts once x_sb space frees)
                for hh in range(WOA_HEADS, REP):
                    for gp in range(GROUP):
                        nc.sync.dma_start(
                            wob_sb[:, hh - WOA_HEADS, gp, :],
                            wor[:, 7 * gp + hh, :],
                        )

                def wsl(hh, gp, m):
                    msl = slice(128 * m, 128 * (m + 1))
                    if hh < WOA_HEADS:
                        return woa_sb[:, hh, gp, msl]
                    return wob_sb[:, hh - WOA_HEADS, gp, msl]

                yacc = yaccp.tile([128, REP, T], FP16, tag="yacc")
                ytr = yt.rearrange("(m p) t -> p m t", p=128)
                for wi, (lo, hi) in enumerate(AGH):
                    hgps = [(hh, gp) for hh in range(lo, hi)
                            for gp in range(GROUP)]
                    for m in range(REP):
                        for n in range(NT):
                            ps = pp6.tile([128, 512], FP32, tag="y",
                                          name=f"y_{wi}_{m}_{n}")
                            for j, (hh, gp) in enumerate(hgps):
                                nc.tensor.matmul(
                                    ps[:],
                                    wsl(hh, gp, m),
                                    otf[:, 7 * gp + hh, 512 * n:512 * (n + 1)],
                                    start=(j == 0),
                                    stop=(j == len(hgps) - 1),
                                )
                            dst = yacc[:, m, 512 * n:512 * (n + 1)]
                            if wi == 0:
                                nc.scalar.copy(dst, ps[:])
                            else:
                                nc.vector.tensor_tensor(dst, dst, ps[:], op=addop)
                        if wi == len(AGH) - 1:
                            nc.scalar.dma_start(ytr[:, m, :], yacc[:, m, :])
            woa_ctx.__exit__(None, None, None)

    nc.compile()
    return nc


def _host_prep(x, segment_ids, Wq, bq, Wk, bk, Wv, bv, Wo):
    """Numpy-side input prep: transpose x, slice weights, RoPE tables, mask."""
    f16 = np.float16
    valid = (segment_ids != 0)
    pos = (np.cumsum(valid, axis=-1) - 1).astype(np.int32)  # CUR_IND = 0
    half = HEAD_DIM // 2
    fraction = np.arange(half, dtype=np.float32) / half
    timescale = ROPE_THETA ** fraction
    ang = pos[..., None].astype(np.float32) / timescale      # (B, T, 64)
    sin = np.sin(ang).astype(np.float32)
    cos = np.cos(ang).astype(np.float32)

    sl = np.arange(128)
    tri = np.where(sl[None, :] >= sl[:, None], 0.0, MASK_VAL).astype(f16)

    in_maps = []
    for c in range(NCORES):
        b, g = c // GROUP, c % GROUP
        qcols = slice(REP * 128 * g, REP * 128 * (g + 1))
        kvcols = slice(128 * g, 128 * (g + 1))
        bias = np.concatenate(
            [bq[qcols].reshape(REP, 128), bk[kvcols][None, :], bv[kvcols][None, :]],
            axis=0,
        ).astype(np.float32)
        sincat = np.concatenate([-sin[b].T, sin[b].T], axis=0)  # (128, T)
        coscat = np.concatenate([cos[b].T, cos[b].T], axis=0)
        in_maps.append({
            "xt": np.ascontiguousarray(x[b].T).astype(f16),
            "wq": np.ascontiguousarray(Wq[:, qcols]).astype(f16),
            "wk": np.ascontiguousarray(Wk[:, kvcols]).astype(f16),
            "wv": np.ascontiguousarray(Wv[:, kvcols]).astype(f16),
            "wo": np.ascontiguousarray(Wo[:, qcols]).astype(f16),
            "bqkv": bias,
            "sincat": np.ascontiguousarray(sincat).astype(f16),
            "coscat": np.ascontiguousarray(coscat).astype(f16),
            "umask": tri,
            "onescol": np.ones((128, 1), f16),
            "onesrow": np.ones((1, 128), f16),
        })
    return in_maps


def _assemble(results):
    y = np.empty((B, T, D), dtype=np.float32)
    for b in range(B):
        blocks = [np.asarray(results[GROUP * b + g]["yt"], dtype=np.float32)
                  for g in range(GROUP)]
        y[b] = np.concatenate(blocks, axis=0).T
    return y


def kernel(x, segment_ids, k_cache, v_cache, Wq, bq, Wk, bk, Wv, bv, Wo,
           _trace=False, _trace_kwargs=None):
    # k_cache/v_cache are zero-initialized and fully overwritten by this
    # prefill (CUR_IND=0, cache_size==T), so they do not affect the output.
    from concourse.bass_utils import run_bass_kernel_spmd

    in_maps = _host_prep(
        np.asarray(x), np.asarray(segment_ids),
        np.asarray(Wq), np.asarray(bq), np.asarray(Wk), np.asarray(bk),
        np.asarray(Wv), np.asarray(bv), np.asarray(Wo),
    )
    if "nc" not in _CACHE:
        _CACHE["nc"] = _build_nc()
    kw = {}
    if _trace:
        kw.update(trace=True, **(_trace_kwargs or {}))
    br = run_bass_kernel_spmd(_CACHE["nc"], in_maps, core_ids=list(range(NCORES)), **kw)
    y = _assemble(br.results)
    if _trace:
        _CACHE["last_result"] = br
    return y


# revision 30
# speedup vs baseline: 1.1190x; 1.1190x over previous
"""GQA causal-attention prefill kernel for 8 Trainium2 NeuronCores.

Sharding: core c -> (batch b = c//4, kv head g = c%4).
Replica groups [[0,1,2,3],[4,5,6,7]] (one per batch).

v3: attention head-blocks interleaved into the projection phase so the
per-head-group AllGathers fire ~80us earlier and the serial CC-ring
chain (~40us per 2-head AG, ~62 GB/s) hides under remaining projection
chains + later attention + o_proj waves. fp16 operands everywhere
(half DMA + SBUF traffic at the same 1 cycle/row PE rate), causal mask
folded into the S PSUM accumulation group as an identity-weights
matmul, v transposed via DMA XBAR (no PE/PSUM), rotating q/wq buffers
to fit SBUF, DMA queues split by role (sync=input stream, vector=og
writes, scalar=wo/yt, gpsimd=oag unpack), fp16 output.

Per-core program order:
  k(n0),k(n1),v(n0),v(n1),q0,q1, [attn h0], q2, [attn h1], q3,
  [attn h2], q4, [attn h3], q5, [attn h4], q6, [attn h5], [attn h6],
  o_proj waves in AG arrival order {h0},{h1,h2},{h3,h4},{h5,h6}.
Output per core: y[b][:, 896g:896(g+1)].T in fp16, host concatenates.
"""
import sys

if '/opt/trn_rl_repo' not in sys.path:
    sys.path.insert(0, '/opt/trn_rl_repo')

import ml_dtypes
import numpy as np

B, T, D = 2, 1024, 3584
NUM_HEADS, HEAD_DIM, NUM_KV = 28, 128, 4
REP = NUM_HEADS // NUM_KV            # 7
ROPE_THETA = 1000000.0
MASK_VAL = -30000.0                  # fp16-representable; *SCALE then exp -> 0
SCALE = HEAD_DIM ** -0.5
GROUP = 4                            # tensor-parallel group size (kv heads)
NCORES = 8
DK = D // 128                        # 28 contraction chunks over D
NT = T // 512                        # token 512-tiles
SK = T // 128                        # key 128-chunks
RG = [[0, 1, 2, 3], [4, 5, 6, 7]]
AGH = [(0, 1), (1, 3), (3, 5), (5, 6), (6, 7)]  # AllGather head groups [lo, hi)
WOA_HEADS = 3              # heads 0..2 (waves 1-2) get early wo weights

_CACHE = {}


def _build_nc():
    """Build the SPMD Bass program (same program on all 8 cores)."""
    import concourse.tile as tile
    from concourse import bacc, mybir
    from concourse.masks import make_identity

    FP32 = mybir.dt.float32
    FP16 = mybir.dt.float16
    Exp = mybir.ActivationFunctionType.Exp
    Ident = mybir.ActivationFunctionType.Identity
    mult = mybir.AluOpType.mult
    addop = mybir.AluOpType.add

    nc = bacc.Bacc("TRN2", target_bir_lowering=False, debug=False, num_devices=NCORES)

    xt = nc.dram_tensor("xt", [D, T], FP16, kind="ExternalInput")
    wq = nc.dram_tensor("wq", [D, REP * 128], FP16, kind="ExternalInput")
    wk = nc.dram_tensor("wk", [D, 128], FP16, kind="ExternalInput")
    wv = nc.dram_tensor("wv", [D, 128], FP16, kind="ExternalInput")
    wo = nc.dram_tensor("wo", [D, REP * 128], FP16, kind="ExternalInput")
    bqkv = nc.dram_tensor("bqkv", [REP + 2, 128], FP32, kind="ExternalInput")
    sincat = nc.dram_tensor("sincat", [128, T], FP16, kind="ExternalInput")
    coscat = nc.dram_tensor("coscat", [128, T], FP16, kind="ExternalInput")
    umask = nc.dram_tensor("umask", [128, 128], FP16, kind="ExternalInput")
    onescol = nc.dram_tensor("onescol", [128, 1], FP16, kind="ExternalInput")
    onesrow = nc.dram_tensor("onesrow", [1, 128], FP16, kind="ExternalInput")
    yt = nc.dram_tensor("yt", [REP * 128, T], FP16, kind="ExternalOutput")

    with tile.TileContext(nc) as tc:
        with (
            tc.tile_pool(name="consts", bufs=1) as consts,
            tc.tile_pool(name="qkv", bufs=1) as qkv,
            tc.tile_pool(name="qp", bufs=3) as qp,
            tc.tile_pool(name="dram", bufs=1, space="DRAM") as dram,
            tc.tile_pool(name="ep", bufs=3) as ep,
            tc.tile_pool(name="otp", bufs=1) as otp,
        ):
            ones_col = consts.tile([128, 1], FP16, tag="onescol")
            ones_row = consts.tile([1, 128], FP16, tag="onesrow")
            bias_sb = consts.tile([128, REP + 2], FP32, tag="bias")
            umask_sb = consts.tile([128, 128], FP16, tag="umask")
            id_sb = consts.tile([128, 128], FP16, tag="ident")
            make_identity(nc, id_sb[:])
            nc.scalar.dma_start(ones_col[:], onescol[:])
            nc.scalar.dma_start(ones_row[:], onesrow[:])
            nc.scalar.dma_start(bias_sb[:], bqkv.rearrange("m p -> p m"))
            nc.scalar.dma_start(umask_sb[:], umask[:])

            k_sb = qkv.tile([128, T], FP16, tag="k")
            vn_sb = qkv.tile([128, SK, 128], FP16, tag="vn")
            otf = otp.tile([128, NUM_HEADS, T], FP16, tag="otf")

            # head-group DRAM blocks for the pipelined AllGather (fp16)
            og = [dram.tile([(hi - lo) * 128, T], FP16, tag=f"og{i}", name=f"og{i}")
                  for i, (lo, hi) in enumerate(AGH)]
            oag = [dram.tile([GROUP * (hi - lo) * 128, T], FP16,
                             tag=f"oag{i}", name=f"oag{i}")
                   for i, (lo, hi) in enumerate(AGH)]

            # warmup collective: absorbs first-op CC-stream setup cost
            wu_in = dram.tile([128, 16], FP16, tag="wuin", name="wuin")
            wu_out = dram.tile([GROUP * 128, 16], FP16, tag="wuout", name="wuout")
            wu_sb = consts.tile([128, 16], FP16, tag="wusb")
            nc.vector.memset(wu_sb[:], 0.0)
            nc.gpsimd.dma_start(wu_in[:], wu_sb[:])
            nc.gpsimd.collective_compute(
                "AllGather",
                mybir.AluOpType.bypass,
                replica_groups=RG,
                ins=[wu_in[:].opt()],
                outs=[wu_out[:].opt()],
            )

            # early o_proj weights: heads 0..WOA_HEADS-1 (all 4 gp), loaded
            # during attention so waves 1-2 never wait; rest after x frees
            woa_ctx = tc.tile_pool(name="woa", bufs=1)
            woa = woa_ctx.__enter__()
            woa_sb = woa.tile([128, WOA_HEADS, GROUP, REP * 128], FP16, tag="woa")
            wor = wo.rearrange("(c p) n -> p c n", p=128)

            # ================= interleaved phase 1 + 4 ===================
            xp_ctx = tc.tile_pool(name="xp", bufs=1)
            xp = xp_ctx.__enter__()
            wqp_ctx = tc.tile_pool(name="wqp", bufs=2)
            wqp = wqp_ctx.__enter__()
            pp1_ctx = tc.tile_pool(name="pp1", bufs=2, space="PSUM")
            pp1 = pp1_ctx.__enter__()
            ropep_ctx = tc.tile_pool(name="ropep", bufs=2)
            ropep = ropep_ctx.__enter__()
            ppatt_ctx = None
            ppatt = None

            sin_sb = xp.tile([128, T], FP16, tag="sin")
            cos_sb = xp.tile([128, T], FP16, tag="cos")
            v_sb = xp.tile([128, T], FP16, tag="v")
            x_sb = xp.tile([128, DK, T], FP16, tag="x")
            wk_sb = xp.tile([128, DK, 128], FP16, tag="wk")
            wv_sb = xp.tile([128, DK, 128], FP16, tag="wv")

            xr = xt.rearrange("(c p) t -> p c t", p=128)
            wqr = wq.rearrange("(c p) n -> p c n", p=128)
            wkr = wk.rearrange("(c p) n -> p c n", p=128)
            wvr = wv.rearrange("(c p) n -> p c n", p=128)

            # priority-ordered input stream: x + k/v weights on the sync
            # queue (first chain fed early), q weights + rope tables on the
            # scalar queue in parallel
            nc.sync.dma_start(wk_sb[:, 0:7, :], wkr[:, 0:7, :])
            nc.sync.dma_start(x_sb[:, 0:7, 0:512], xr[:, 0:7, 0:512])
            nc.sync.dma_start(wk_sb[:, 7:DK, :], wkr[:, 7:DK, :])
            for quarter in range(1, 4):
                sl = (slice(None), slice(7 * quarter, 7 * quarter + 7),
                      slice(0, 512))
                nc.sync.dma_start(x_sb[sl], xr[sl])
            nc.sync.dma_start(wv_sb[:], wvr[:])
            for quarter in range(4):
                sl = (slice(None), slice(7 * quarter, 7 * quarter + 7),
                      slice(512, 1024))
                nc.sync.dma_start(x_sb[sl], xr[sl])
            nc.scalar.dma_start(sin_sb[:], sincat[:])
            nc.scalar.dma_start(cos_sb[:], coscat[:])

            wq_tiles = {}

            def load_wq(h, eng=None):
                wt = wqp.tile([128, DK, 128], FP16, tag="wqh", name=f"wq_{h}")
                (eng or nc.sync).dma_start(wt[:], wqr[:, :, 128 * h:128 * (h + 1)])
                wq_tiles[h] = wt

            load_wq(0, nc.scalar)
            load_wq(1, nc.scalar)

            def rope(X_full, n):
                X = X_full[:, 512 * n:512 * (n + 1)]
                tmp = ropep.tile([128, 512], FP16, tag="ropetmp")
                nc.vector.tensor_copy(tmp[0:64, :], X[64:128, :])
                nc.vector.tensor_copy(tmp[64:128, :], X[0:64, :])
                ssl = (slice(None), slice(512 * n, 512 * (n + 1)))
                nc.vector.tensor_tensor(tmp[:], tmp[:], sin_sb[ssl], op=mult)
                nc.vector.tensor_tensor(X, X, cos_sb[ssl], op=mult)
                nc.vector.tensor_tensor(X, X, tmp[:], op=addop)

            def chain(wsl3, dst, bi, n):
                """One projection chain: dst[:,512n:+512] = (w.T @ x) + bias."""
                ps = pp1.tile([128, 512], FP32, tag="proj", name=f"proj_{bi}_{n}")
                for kc in range(DK):
                    nc.tensor.matmul(
                        ps[:],
                        wsl3[:, kc, :],
                        x_sb[:, kc, 512 * n:512 * (n + 1)],
                        start=(kc == 0),
                        stop=(kc == DK - 1),
                    )
                nc.scalar.activation(
                    dst[:, 512 * n:512 * (n + 1)], ps[:], Ident,
                    bias=bias_sb[:, bi:bi + 1], scale=1.0,
                )

            # ---- k, v projections (+rope / PE transposes) ----
            # transposes use a transient PSUM pool that closes before the
            # attention pool opens (PSUM stays within 8 banks)
            trp_ctx = tc.tile_pool(name="trp", bufs=2, space="PSUM")
            trp = trp_ctx.__enter__()
            # n-interleaved so the n=0 work rides out the x n=1 DMA arrival
            for n in range(NT):
                chain(wk_sb, k_sb, 7, n)
                rope(k_sb, n)
                chain(wv_sb, v_sb, 8, n)
                for sc in range(4 * n, 4 * n + 4):
                    tp = trp.tile([128, 128], FP16, tag="tr", name=f"tr_{sc}")
                    nc.tensor.transpose(
                        tp[:], v_sb[:, 128 * sc:128 * (sc + 1)], id_sb[:]
                    )
                    nc.scalar.copy(vn_sb[:, sc, :], tp[:])
            trp_ctx.__exit__(None, None, None)
            ppatt_ctx = tc.tile_pool(name="ppatt", bufs=1, space="PSUM")
            ppatt = ppatt_ctx.__enter__()
            # early wo stream (sync queue)
            for hh in range(WOA_HEADS):
                for gp in range(GROUP):
                    nc.sync.dma_start(
                        woa_sb[:, hh, gp, :], wor[:, 7 * gp + hh, :]
                    )

            q_tiles = {}

            def qchain(h):
                qt = qp.tile([128, T], FP16, tag="q", name=f"q_{h}")
                for n in range(NT):
                    chain(wq_tiles[h], qt, h, n)
                    rope(qt, n)
                del wq_tiles[h]
                if h + 2 < REP:
                    load_wq(h + 2)
                q_tiles[h] = qt

            # ---- attention block for one head ----
            pending = []

            def finalize(h, tau, den, ops):
                rec = ep.tile([1, 512], FP32, tag="rec", name=f"rec_{h}_{tau}")
                nc.vector.reciprocal_approx_fast(rec[:], den[0:1, :])
                rec16 = ep.tile([1, 512], FP16, tag="rec16", name=f"rec16_{h}_{tau}")
                nc.vector.tensor_copy(rec16[:], rec[:])
                bc = ppatt.tile([128, 512], FP32, tag="den",
                                name=f"bc_{h}_{tau}")
                nc.tensor.matmul(bc[:], ones_row[:], rec16[:], start=True, stop=True)
                bcs = ep.tile([128, 512], FP16, tag="bcs", name=f"bcs_{h}_{tau}")
                nc.scalar.copy(bcs[:], bc[:])
                ost = ep.tile([128, 512], FP16, tag="ost", name=f"ost_{h}_{tau}")
                nc.vector.tensor_tensor(ost[:], ops[:], bcs[:], op=mult)
                grp = next(i for i, (lo, hi) in enumerate(AGH) if lo <= h < hi)
                lo, hi = AGH[grp]
                nc.sync.dma_start(
                    og[grp][128 * (h - lo):128 * (h - lo + 1),
                            512 * tau:512 * (tau + 1)],
                    ost[:],
                )
                if tau == NT - 1 and h == hi - 1:
                    nc.gpsimd.collective_compute(
                        "AllGather",
                        mybir.AluOpType.bypass,
                        replica_groups=RG,
                        ins=[og[grp][:].opt()],
                        outs=[oag[grp][:].opt()],
                    )
                    nh = hi - lo
                    for hh in range(lo, hi):
                        for gp in range(GROUP):
                            r0 = nh * 128 * gp + 128 * (hh - lo)
                            nc.gpsimd.dma_start(
                                otf[:, 7 * gp + hh, :],
                                oag[grp][r0:r0 + 128, :],
                            )

            def attn(h):
                qt = q_tiles.pop(h)
                for tau in range(NT):
                    n_sc = 4 * (tau + 1)
                    den = ppatt.tile([128, 512], FP32, tag="den",
                                     name=f"den_{h}_{tau}")[0:1, :]
                    ops = ppatt.tile([128, 512], FP32, tag=f"opv{tau % 2}",
                                     name=f"ops_{h}_{tau}")
                    esum = ep.tile([128, 512], FP16, tag="esum",
                                   name=f"esum_{h}_{tau}")
                    etiles = {}

                    def emit_s(c):
                        delta = 128 * c - 512 * tau
                        t0 = max(delta, 0)
                        w = 512 - t0
                        sps = ppatt.tile([128, 512], FP32, tag=f"s{c % 3}",
                                         name=f"sps_{h}_{tau}_{c}")
                        tsl = slice(512 * tau + t0, 512 * (tau + 1))
                        nc.tensor.matmul(
                            sps[:, 0:w],
                            k_sb[:, 128 * c:128 * (c + 1)],
                            qt[:, tsl],
                            start=True,
                            stop=(delta < 0),
                            skip_group_check=True,
                        )
                        if delta >= 0:
                            # causal mask add on the diagonal 128 block:
                            # id.T @ umask == umask, accumulated in-group
                            nc.tensor.matmul(
                                sps[:, 0:128],
                                id_sb[:],
                                umask_sb[:],
                                start=False,
                                stop=True,
                                skip_group_check=True,
                            )
                        et = ep.tile([128, 512], FP16, tag="e",
                                     name=f"et_{h}_{tau}_{c}")
                        nc.scalar.activation(et[:, 0:w], sps[:, 0:w], Exp, scale=SCALE)
                        etiles[c] = (et, t0, w)

                    def emit_acc(c):
                        et, t0, w = etiles.pop(c)
                        # E column-sum accumulates on DVE (fp16 2x) instead
                        # of a PE ones-matmul chain
                        if c == 0:
                            nc.vector.tensor_copy(esum[:], et[:])
                        else:
                            nc.vector.tensor_tensor(
                                esum[:, t0:512], esum[:, t0:512], et[:, 0:w],
                                op=addop,
                            )
                        nc.tensor.matmul(
                            ops[:, t0:512], vn_sb[:, c, :], et[:, 0:w],
                            start=(c == 0), stop=(c == n_sc - 1),
                        )

                    LOOKAHEAD = 2
                    for c in range(n_sc):
                        emit_s(c)
                        if c == LOOKAHEAD and pending:
                            finalize(*pending.pop(0))
                        if c >= LOOKAHEAD:
                            emit_acc(c - LOOKAHEAD)
                    for c in range(max(0, n_sc - LOOKAHEAD), n_sc):
                        emit_acc(c)
                    # single PE matmul turns esum into the softmax denominator
                    nc.tensor.matmul(
                        den[0:1, :], ones_col[:], esum[:], start=True, stop=True
                    )
                    pending.append((h, tau, den, ops))

            # ---- interleaved schedule ----
            qchain(0)
            qchain(1)
            for h in range(REP):
                attn(h)
                if h + 2 < REP:
                    qchain(h + 2)
                if pending:
                    finalize(*pending.pop(0))
            while pending:
                finalize(*pending.pop(0))

            ppatt_ctx.__exit__(None, None, None)
            ropep_ctx.__exit__(None, None, None)
            pp1_ctx.__exit__(None, None, None)
            wqp_ctx.__exit__(None, None, None)
            xp_ctx.__exit__(None, None, None)

            # ---- Phase 6: o_proj as per-AG-wave partial sums ------------
            with (
                tc.tile_pool(name="wob", bufs=1) as wob,
                tc.tile_pool(name="yaccp", bufs=1) as yaccp,
                tc.tile_pool(name="pp6", bufs=3, space="PSUM") as pp6,
            ):
                nwob = REP - WOA_HEADS
                wob_sb = wob.tile([128, nwob, GROUP, REP * 128], FP16, tag="wob")
                # late wo stream (sync queue, starts once x_sb space frees)
                for hh in range(WOA_HEADS, REP):
                    for gp in range(GROUP):
                        nc.sync.dma_start(
                            wob_sb[:, hh - WOA_HEADS, gp, :],
                            wor[:, 7 * gp + hh, :],
                        )

                def wsl(hh, gp, m):
                    msl = slice(128 * m, 128 * (m + 1))
                    if hh < WOA_HEADS:
                        return woa_sb[:, hh, gp, msl]
                    return wob_sb[:, hh - WOA_HEADS, gp, msl]

                yacc = yaccp.tile([128, REP, T], FP16, tag="yacc")
                ytr = yt.rearrange("(m p) t -> p m t", p=128)
                for wi, (lo, hi) in enumerate(AGH):
                    hgps = [(hh, gp) for hh in range(lo, hi)
                            for gp in range(GROUP)]
                    for m in range(REP):
                        for n in range(NT):
                            ps = pp6.tile([128, 512], FP32, tag="y",
                                          name=f"y_{wi}_{m}_{n}")
                            for j, (hh, gp) in enumerate(hgps):
                                nc.tensor.matmul(
                                    ps[:],
                                    wsl(hh, gp, m),
                                    otf[:, 7 * gp + hh, 512 * n:512 * (n + 1)],
                                    start=(j == 0),
                                    stop=(j == len(hgps) - 1),
                                )
                            dst = yacc[:, m, 512 * n:512 * (n + 1)]
                            if wi == 0:
                                nc.scalar.copy(dst, ps[:])
                            else:
                                nc.vector.tensor_tensor(dst, dst, ps[:], op=addop)
                        if wi == len(AGH) - 1:
                            nc.scalar.dma_start(ytr[:, m, :], yacc[:, m, :])
            woa_ctx.__exit__(None, None, None)

    nc.compile()
    return nc


def _host_prep(x, segment_ids, Wq, bq, Wk, bk, Wv, bv, Wo):
    """Numpy-side input prep: transpose x, slice weights, RoPE tables, mask."""
    f16 = np.float16
    valid = (segment_ids != 0)
    pos = (np.cumsum(valid, axis=-1) - 1).astype(np.int32)  # CUR_IND = 0
    half = HEAD_DIM // 2
    fraction = np.arange(half, dtype=np.float32) / half
    timescale = ROPE_THETA ** fraction
    ang = pos[..., None].astype(np.float32) / timescale      # (B, T, 64)
    sin = np.sin(ang).astype(np.float32)
    cos = np.cos(ang).astype(np.float32)

    sl = np.arange(128)
    tri = np.where(sl[None, :] >= sl[:, None], 0.0, MASK_VAL).astype(f16)

    in_maps = []
    for c in range(NCORES):
        b, g = c // GROUP, c % GROUP
        qcols = slice(REP * 128 * g, REP * 128 * (g + 1))
        kvcols = slice(128 * g, 128 * (g + 1))
        bias = np.concatenate(
            [bq[qcols].reshape(REP, 128), bk[kvcols][None, :], bv[kvcols][None, :]],
            axis=0,
        ).astype(np.float32)
        sincat = np.concatenate([-sin[b].T, sin[b].T], axis=0)  # (128, T)
        coscat = np.concatenate([cos[b].T, cos[b].T], axis=0)
        in_maps.append({
            "xt": np.ascontiguousarray(x[b].T).astype(f16),
            "wq": np.ascontiguousarray(Wq[:, qcols]).astype(f16),
            "wk": np.ascontiguousarray(Wk[:, kvcols]).astype(f16),
            "wv": np.ascontiguousarray(Wv[:, kvcols]).astype(f16),
            "wo": np.ascontiguousarray(Wo[:, qcols]).astype(f16),
            "bqkv": bias,
            "sincat": np.ascontiguousarray(sincat).astype(f16),
            "coscat": np.ascontiguousarray(coscat).astype(f16),
            "umask": tri,
            "onescol": np.ones((128, 1), f16),
            "onesrow": np.ones((1, 128), f16),
        })
    return in_maps


def _assemble(results):
    y = np.empty((B, T, D), dtype=np.float32)
    for b in range(B):
        blocks = [np.asarray(results[GROUP * b + g]["yt"], dtype=np.float32)
                  for g in range(GROUP)]
        y[b] = np.concatenate(blocks, axis=0).T
    return y


def kernel(x, segment_ids, k_cache, v_cache, Wq, bq, Wk, bk, Wv, bv, Wo,
           _trace=False, _trace_kwargs=None):
    # k_cache/v_cache are zero-initialized and fully overwritten by this
    # prefill (CUR_IND=0, cache_size==T), so they do not affect the output.
    from concourse.bass_utils import run_bass_kernel_spmd

    in_maps = _host_prep(
        np.asarray(x), np.asarray(segment_ids),
        np.asarray(Wq), np.asarray(bq), np.asarray(Wk), np.asarray(bk),
        np.asarray(Wv), np.asarray(bv), np.asarray(Wo),
    )
    if "nc" not in _CACHE:
        _CACHE["nc"] = _build_nc()
    kw = {}
    if _trace:
        kw.update(trace=True, **(_trace_kwargs or {}))
    br = run_bass_kernel_spmd(_CACHE["nc"], in_maps, core_ids=list(range(NCORES)), **kw)
    y = _assemble(br.results)
    if _trace:
        _CACHE["last_result"] = br
    return y
